# revision 55
# baseline (speedup 1.0000x reference)
"""Trainium2 Bass kernel for nn_CostFn_18562848653837 (v3).

reference(x, cond, time) only reads x[b, j, 6+k] for j in [0,26), k in [0,6)
(~2.6 MB of the 436 MB input; cond/time are unused) and computes, per point,
the reflected mass 1 / (u^T J M^{-1} J^T u) with u = e_x, which reduces via
Sherman-Morrison (M = 2I + 0.5 c c^T, c = cos(cq), s = sin(cq),
cq = cumsum(q)) to

    cost = -2*TC / (TB - G)
    TC = 1.75 + C0/8;   C0 = sum_k cos(2 cq_k)
    Q1 = 1.355 - C2/2;  C2 = sum_k L_k^2 cos(2 cq_k);  G = Q1*TC
    TB = P2^2/16;       P2 = sum_k L_k sin(2 cq_k)

i.e. everything depends only on sin/cos of 2*cq. Host-side input prep
computes the cumulative angles and wraps them into Sin-table range:
m = frac(cq/pi) and m2 = frac(cq/pi + 1/4), so on device
sin(2*pi*m) = sin(2cq) and sin(2*pi*m2) = cos(2cq) -- cos via the
quarter-turn shift, no second activation table.

Device pipeline (per core, 13312 points as (128, 104 w, 6 k) fp16 k-minor,
input tile [m | m2] = (128, 1248)):
 - ACT: four chunked Sins (CS-A, CS-B, SF-A, SF-B; chunk split at point
   72) so downstream vector work starts after the first cos chunk instead
   of the full tile; the dep-free warm-up Sin hoists the ~1.3us table
   load to t~0 in parallel with the input DMA. ACT is busy 1483..3263 and
   is the pipeline's pacing engine.
 - weighted k-sums: DVE tensor_tensor_scan with ratio patterns (Horner
   form, L5^p folded into the final affine consts) for its point share
   (C2: 56/28, P2: 54/26 per chunk); Pool covers the rest with strided
   per-k ops and computes C0 for all points (5 strided adds/chunk). The
   v1 cost model charges Pool a flat 0.833 ns/element with no efficiency
   penalty, so Pool also runs ALL the per-point smalls (TCC/Q1C/GC/TB/DN)
   except the chunk-B square, which stays on DVE to keep the closing
   chain free of cross-engine hops.
 - finale per chunk: reciprocal + affine_mul_reduce on DVE -> per-
   partition partials in COL[:, chunk]. (tt-divide would be one op
   cheaper but is not a valid DVE ISA op on HW.)
 - output without any DMA: PE ones-matmuls accumulate both COL columns
   into one PSUM scalar; DVE copies it to SBUF, TENSOR_LOADs it into a
   register and TENSOR_SAVEs the 4 bytes straight to the DRAM output.
   The epilogue then has no DMA-queue latency to drain (saves ~2.3 us vs
   a dma_start of the partials: the hwdge drain charges queue-end +
   1717 ns and the epilogue sem-wait blocks on it).

Sharding: pure data parallel over batch - core i gets batches
[512*i, 512*(i+1)); host adds the 8 per-core scalars.
"""

import numpy as np

_P, _W, _K = 128, 104, 6
_F = _K * _W          # 624
_NCORES = 8
_B, _H, _T = 4096, 1024, 26
_BPC = _B // _NCORES  # batches per core

# chunk sizes (points): A computed first on ACT, B second
_PA = 72
_PB = _W - _PA
# DVE point-share of each scan stage per chunk (rest on Pool as strided ops)
_C2A = 56
_C2B = 28
# tail (SF/P2S) chunk boundary and DVE shares -- decoupled from the mid chunk
_TA = 72
_P2A = 54
_P2B = 26
# ACT op order: (tile 0=CS/1=SF, lo_pt, hi_pt)
_ACT_ORDER = [(0, 0, _PA), (0, _PA, _W), (1, 0, _TA), (1, _TA, _W)]

_CACHE = {}


def _get_nc():
    if "nc" in _CACHE:
        return _CACHE["nc"]

    import concourse.tile as tile
    import concourse.mybir as mybir
    from concourse import bacc

    # One-ulp-shaded 2*pi: |m| <= 0.5 exactly, so the fp16-rounded Sin input
    # |SCALE2*m| stays inside the [-pi, pi] table domain unconditionally.
    SCALE2 = float(np.float32(2.0 * np.pi * (1.0 - 2.0**-23)))
    L = np.arange(1, 7, dtype=np.float32) * np.float32(0.1) + np.float32(0.3)
    L5SQ = float(np.float32(L[5] * L[5]))
    RHO1 = [0.0] + [float(np.float32(L[k - 1] / L[k]) ** 2) for k in range(1, _K)]
    RHO2 = [0.0] + [float(np.float32(L[k - 1] / L[k])) for k in range(1, _K)]
    # finale rescaled by 16/L5^2 so TB needs no scale op:
    #   Q1' = 16*1.355/L5^2 - 8*C2S ; G' = Q1'*TC ; TB' = P2S^2
    #   cost = (-32/L5^2) * TC/(TB' - G')
    Q1_B = float(np.float32(16.0 * 1.355 / L5SQ))
    Q1_A = -8.0
    AMRSCALE = float(np.float32(-32.0 / L5SQ))

    f32 = mybir.dt.float32
    f16 = mybir.dt.float16
    i32 = mybir.dt.int32
    OP = mybir.AluOpType
    ACT = mybir.ActivationFunctionType

    nc = bacc.Bacc(
        "TRN2", target_bir_lowering=False, debug=False, num_devices=_NCORES,
        disable_frame_to_traceback=True,
    )
    q_dram = nc.dram_tensor("q", [_P, 2 * _F], f16, kind="ExternalInput")
    out_dram = nc.dram_tensor("out", [1, 1], f32, kind="ExternalOutput")

    # column boundaries
    cA0, cA1 = 0, 6 * _PA                # chunk A cols in the 624 layout
    cB0, cB1 = 6 * _PA, _F

    kv = lambda t, kk, lo, hi: t[:].rearrange(
        "p (w k) -> p w k", k=_K
    )[:, lo:hi, kk]

    with (
        tile.TileContext(nc) as tc,
        tc.tile_pool(name="pool", bufs=1) as pool,
        tc.psum_pool(name="psc_pool", bufs=1) as psum_pool,
        nc.allow_low_precision(reason="fp16 pipeline validated to 3e-5"),
    ):
        v = nc.vector   # DVE
        g = nc.gpsimd   # Pool
        a = nc.scalar   # ACT

        PSC = psum_pool.tile([_P, 2], f32)
        QT = pool.tile([_P, 2 * _F], f16)   # [m | m2]
        R1T = pool.tile([_P, _F], f16)
        R2T = pool.tile([_P, _F], f16)
        CS = pool.tile([_P, _F], f16)       # cos(2cq) = sin(2pi m2)
        SF = pool.tile([_P, _F], f16)       # sin(2cq)
        C2S = pool.tile([_P, _F], f16)
        P2S = pool.tile([_P, _F], f16)
        C0P = pool.tile([_P, _W], f16)      # per-point C0 (Pool, all points)
        C2P = pool.tile([_P, _W], f16)
        P2P = pool.tile([_P, _W], f16)
        TCC = pool.tile([_P, _W], f16)
        Q1C = pool.tile([_P, _W], f16)
        GC = pool.tile([_P, _W], f16)
        TBC = pool.tile([_P, _W], f16)
        DNC = pool.tile([_P, _W], f16)
        DIVR = pool.tile([_P, _W], f16)
        AMRO = pool.tile([_P, _W], f16)
        COL = pool.tile([_P, 2], f32)
        WARM = pool.tile([_P, 1], f32)
        RES = pool.tile([_P, 1], f32)

        # --- input DMA: one (128, 2496B/partition) transfer on the SP queue.
        nc.sync.dma_start(QT[:], q_dram[:])

        # Dep-free warm-up Sin: hoists the ~1.3us activation table load to
        # t~0, off the critical path.
        one_ap = nc.const_aps.aps[(f32, 1.0)]
        a.activation(WARM[:], one_ap[:_P], ACT.Sin)

        # --- ACT: chunked Sins. Order is tunable via _ACT_ORDER: each entry
        # is (tile, lo, hi) with tile 0=CS (cos, input m2) / 1=SF (sin, m).
        for which, lo, hi in _ACT_ORDER:
            dst = CS if which == 0 else SF
            off = _F if which == 0 else 0
            a.activation(
                dst[:, 6 * lo : 6 * hi], QT[:, off + 6 * lo : off + 6 * hi],
                ACT.Sin, scale=SCALE2,
            )

        # --- Pool preamble: ratio patterns for the DVE Horner scans (only
        # the DVE column shares are read; fill contiguous covers).
        for k in range(_K):
            g.memset(kv(R1T, k, 0, _PA + _C2B), RHO1[k])
        for k in range(_K):
            g.memset(kv(R2T, k, 0, _TA + _P2B), RHO2[k])

        # helper: Pool Horner weighted k-sum over point range [lo, hi)
        def pool_horner(dst, src, rho, lo, hi):
            g.tensor_scalar(dst[:, lo:hi], kv(src, 0, lo, hi), rho[1], None, OP.mult)
            g.tensor_add(dst[:, lo:hi], dst[:, lo:hi], kv(src, 1, lo, hi))
            for k in range(2, _K):
                g.tensor_scalar(dst[:, lo:hi], dst[:, lo:hi], rho[k], None, OP.mult)
                g.tensor_add(dst[:, lo:hi], dst[:, lo:hi], kv(src, k, lo, hi))

        # ===== DVE: the four scan shares, then the per-chunk finale =====
        v.tensor_tensor_scan(
            C2S[:, 0 : 6 * _C2A], R1T[:, 0 : 6 * _C2A], CS[:, 0 : 6 * _C2A],
            0.0, OP.mult, OP.add,
        )
        v.tensor_tensor_scan(
            C2S[:, cB0 : cB0 + 6 * _C2B], R1T[:, cB0 : cB0 + 6 * _C2B],
            CS[:, cB0 : cB0 + 6 * _C2B], 0.0, OP.mult, OP.add,
        )
        # ===== Pool: C0, Horner shares, and ALL the smalls =====
        # --- mid A (needs CS-A)
        g.tensor_add(C0P[:, 0:_PA], kv(CS, 0, 0, _PA), kv(CS, 1, 0, _PA))
        for k in range(2, _K):
            g.tensor_add(C0P[:, 0:_PA], C0P[:, 0:_PA], kv(CS, k, 0, _PA))
        pool_horner(C2P, CS, RHO1, _C2A, _PA)
        # TC = 1.75 + C0/8 ; Q1' = Q1_B + Q1_A*C2S ; G' = Q1'*TC
        g.tensor_scalar(TCC[:, 0:_PA], C0P[:, 0:_PA], 0.125, 1.75, OP.mult, OP.add)
        g.tensor_scalar(Q1C[:, 0:_C2A], kv(C2S, 5, 0, _C2A), Q1_A, Q1_B, OP.mult, OP.add)
        g.tensor_scalar(Q1C[:, _C2A:_PA], C2P[:, _C2A:_PA], Q1_A, Q1_B, OP.mult, OP.add)
        g.tensor_mul(GC[:, 0:_PA], Q1C[:, 0:_PA], TCC[:, 0:_PA])
        # --- mid B (needs CS-B)
        g.tensor_add(C0P[:, _PA:_W], kv(CS, 0, _PA, _W), kv(CS, 1, _PA, _W))
        for k in range(2, _K):
            g.tensor_add(C0P[:, _PA:_W], C0P[:, _PA:_W], kv(CS, k, _PA, _W))
        pool_horner(C2P, CS, RHO1, _PA + _C2B, _W)
        g.tensor_scalar(TCC[:, _PA:_W], C0P[:, _PA:_W], 0.125, 1.75, OP.mult, OP.add)
        g.tensor_scalar(
            Q1C[:, _PA : _PA + _C2B], kv(C2S, 5, _PA, _PA + _C2B), Q1_A, Q1_B,
            OP.mult, OP.add,
        )
        g.tensor_scalar(
            Q1C[:, _PA + _C2B : _W], C2P[:, _PA + _C2B : _W], Q1_A, Q1_B,
            OP.mult, OP.add,
        )
        g.tensor_mul(GC[:, _PA:_W], Q1C[:, _PA:_W], TCC[:, _PA:_W])
        v.tensor_tensor_scan(
            P2S[:, 0 : 6 * _P2A], R2T[:, 0 : 6 * _P2A], SF[:, 0 : 6 * _P2A],
            0.0, OP.mult, OP.add,
        )
        v.tensor_tensor_scan(
            P2S[:, 6 * _TA : 6 * (_TA + _P2B)], R2T[:, 6 * _TA : 6 * (_TA + _P2B)],
            SF[:, 6 * _TA : 6 * (_TA + _P2B)], 0.0, OP.mult, OP.add,
        )
        # --- tail 1 (needs SF-1, pts [0:_TA)): TB' = P2S^2 ; DN' = TB' - G'
        pool_horner(P2P, SF, RHO2, _P2A, _TA)
        g.tensor_mul(TBC[:, 0:_P2A], kv(P2S, 5, 0, _P2A), kv(P2S, 5, 0, _P2A))
        g.tensor_mul(TBC[:, _P2A:_TA], P2P[:, _P2A:_TA], P2P[:, _P2A:_TA])
        g.tensor_sub(DNC[:, 0:_PA], TBC[:, 0:_PA], GC[:, 0:_PA])
        # --- tail 2 (needs SF-2, pts [_TA:_W)): Pool does its Horner share;
        # the DVE-share square runs on DVE (no cross-engine hop on the
        # closing chain).
        pool_horner(P2P, SF, RHO2, _TA + _P2B, _W)
        g.tensor_mul(TBC[:, _TA + _P2B : _W], P2P[:, _TA + _P2B : _W],
                     P2P[:, _TA + _P2B : _W])
        v.scalar_tensor_tensor(
            TBC[:, _TA : _TA + _P2B], kv(P2S, 5, _TA, _TA + _P2B), 1.0,
            kv(P2S, 5, _TA, _TA + _P2B), OP.mult, OP.mult,
        )

        # chunk-A finale fills the wait for Pool's GC-B delivery, then the
        # B chain closes: WREC = 1/DN' ; COL[:,c] = sum (WREC*AMRSCALE)*TC
        # (tt-divide is not a valid DVE ISA op on HW, so reciprocal+AMR).
        v.reciprocal(DIVR[:, 0:_PA], DNC[:, 0:_PA])
        v.affine_mul_reduce(
            AMRO[:, 0:_PA], COL[:, 0:1], DIVR[:, 0:_PA], TCC[:, 0:_PA],
            AMRSCALE, 0.0,
        )
        v.tensor_sub(DNC[:, _PA:_W], TBC[:, _PA:_W], GC[:, _PA:_W])
        v.reciprocal(DIVR[:, _PA:_W], DNC[:, _PA:_W])
        v.affine_mul_reduce(
            AMRO[:, _PA:_W], COL[:, 1:2], DIVR[:, _PA:_W], TCC[:, _PA:_W],
            AMRSCALE, 0.0,
        )

        # --- DMA-less output: PE accumulates both chunk partial-columns into
        # one PSUM scalar; DVE stages it to SBUF and register-stores the 4
        # bytes to DRAM.
        nc.tensor.matmul(PSC[:1, :1], one_ap[:_P], COL[:, 0:1], start=True, stop=False)
        nc.tensor.matmul(PSC[:1, :1], one_ap[:_P], COL[:, 1:2], start=False, stop=True)
        v.tensor_scalar(RES[0:1, 0:1], PSC[0:1, 0:1], 1.0, None, OP.mult)
        res_reg = nc.alloc_register(mybir.EngineType.DVE, "res")
        v.load(res_reg, RES[0:1, 0:1].bitcast(i32))
        v.store(out_dram[0:1, 0:1].bitcast(i32), res_reg)

    nc.compile()
    _CACHE["nc"] = nc
    return nc


def _shard(x):
    # gather the used slice, cumulative angles wrapped into Sin-table range
    q = np.asarray(x[:, :_T, 6 : 6 + _K], dtype=np.float32)
    gpi = np.cumsum(q, axis=-1) * np.float32(1.0 / np.pi)
    m = gpi - np.round(gpi)
    g2 = gpi + np.float32(0.25)
    m2 = g2 - np.round(g2)
    m = m.astype(np.float16).reshape(_NCORES, _P, _F)
    m2 = m2.astype(np.float16).reshape(_NCORES, _P, _F)
    return np.ascontiguousarray(np.concatenate([m, m2], axis=2))


def _get_runner():
    """Build the jitted 8-core shard_map executable once (mirrors
    bass2jax.run_bass_via_pjrt's multi-core path) so repeat kernel() calls
    skip retracing/recompiling."""
    if "run" in _CACHE:
        return _CACHE["run"]
    import jax
    from jax.sharding import Mesh, PartitionSpec
    from jax.experimental.shard_map import shard_map
    from concourse import bass2jax

    nc = _get_nc()
    bass2jax.install_neuronx_cc_hook()
    assert nc.dbg_addr is None
    pid_name = nc.partition_id_tensor.name if nc.partition_id_tensor else None
    in_names = ("q", "out") + ((pid_name,) if pid_name else ())

    out_aval = jax.core.ShapedArray((1, 1), np.float32)

    def _body(q, out_zero):
        operands = [q, out_zero]
        if pid_name is not None:
            operands.append(bass2jax.partition_id_tensor())
        (out,) = bass2jax._bass_exec_p.bind(
            *operands,
            out_avals=(out_aval,),
            in_names=in_names,
            out_names=("out",),
            lowering_input_output_aliases=(),
            sim_require_finite=True,
            sim_require_nnan=True,
            nc=nc,
        )
        return (out,)

    devices = jax.devices()[:_NCORES]
    mesh = Mesh(np.asarray(devices), ("core",))
    sharded = jax.jit(
        shard_map(
            _body,
            mesh=mesh,
            in_specs=(PartitionSpec("core"),) * 2,
            out_specs=(PartitionSpec("core"),),
            check_rep=False,
        ),
        donate_argnums=(1,),
        keep_unused=True,
    )

    def run(planes):
        concat_q = planes.reshape(_NCORES * _P, 2 * _F)
        zeros = np.zeros((_NCORES * 1, 1), np.float32)
        (out,) = sharded(concat_q, zeros)
        return np.asarray(out)  # (8, 1)

    _CACHE["run"] = run
    return run


def _run_library(planes):
    from concourse.bass_utils import run_bass_kernel_spmd

    res = run_bass_kernel_spmd(
        _get_nc(),
        [{"q": planes[i]} for i in range(_NCORES)],
        list(range(_NCORES)),
    )
    return np.stack([r["out"][:, 0] for r in res.results]).astype(np.float32)


def _run_subprocess(planes):
    """Last resort: the accelerator occasionally reports
    NRT_EXEC_UNIT_UNRECOVERABLE; a fresh process reliably recovers it."""
    import os
    import subprocess
    import sys
    import tempfile

    d = tempfile.mkdtemp()
    inp = os.path.join(d, "planes.npy")
    out = os.path.join(d, "out.npy")
    np.save(inp, planes)
    here = os.path.dirname(os.path.abspath(__file__))
    script = (
        "import sys, numpy as np\n"
        f"sys.path.insert(0, {here!r})\n"
        "import kernel as K\n"
        f"planes = np.load({inp!r})\n"
        "out = K._get_runner()(planes)\n"
        f"np.save({out!r}, out)\n"
    )
    err = None
    for _ in range(2):
        try:
            subprocess.run(
                [sys.executable, "-c", script], check=True, timeout=900,
                stdout=subprocess.DEVNULL, stderr=subprocess.DEVNULL,
            )
            return np.load(out).astype(np.float32)
        except Exception as e:  # retry once; device usually recovers
            err = e
    raise err


def kernel(x, cond, time):
    x = np.asarray(x)
    planes = _shard(x)
    try:
        partials = _get_runner()(planes).astype(np.float32)
    except Exception:
        try:
            # library SPMD runner (covers fast-path/jax API drift)
            partials = _run_library(planes)
        except Exception:
            # fresh process recovers a wedged accelerator
            partials = _run_subprocess(planes)
    return np.float32(partials.sum(dtype=np.float32))
